# revision 1
# baseline (speedup 1.0000x reference)
"""Trainium2 Bass kernel for CLSProcess: diagonal linear recurrence
state_t = y_t * state_{t-1} + x_t * z_t over [B=8, T=4096, units=1024].

Sharding: batch across the 8 cores (one batch element per core); the
recurrence is handled per-core with a chunked scan:
  - time is cut into 32 blocks of L=128 steps (partition dim = time)
  - per block, the decay matrix M[t,s] = prod_{r=s+1..t} y_r (0 for s>t)
    is built EXACTLY with a DVE tensor_tensor_scan over the identity:
    state_s(t) = y_t*state + I[s==t]  =>  out[s,t] = M[t,s] (the lhsT
    layout the PE matmul wants). Scans are batched 4 blocks per
    instruction ([128,512]) with the y at block boundaries zeroed so the
    running state resets at each block start.
  - block output = M @ (x*z)  (PE matmul, bf16 operands, fp32 PSUM) +
    carry term
  - carry term: engines can only address partition bases {0,32,64,96},
    so instead of extracting row 127 of the previous block we build
    sel[s,t] = I[s==127] * p_t  (p_t = prod_{r=block_start..t} y_r
    = y_0 * M[t,0], broadcast via GPSIMD + mask on DVE) and accumulate
    sel^T @ prev_out into the same PSUM (float32r single-pass matmul),
    which equals p_t * prev_state.
"""

import numpy as np

import concourse.bacc as bacc
import concourse.bass as bass
import concourse.mybir as mybir
import concourse.tile as tile
from concourse.bass_utils import run_bass_kernel_spmd

B = 8
T = 4096
F = 1026
U = 1024
L = 128
G = 4  # blocks per scan batch
f32 = mybir.dt.float32
f32r = mybir.dt.float32r
bf16 = mybir.dt.bfloat16


def build_nc(t_total: int = T) -> bass.Bass:
    nb = t_total // L
    ng = (nb + G - 1) // G
    nc = bacc.Bacc()
    inp = nc.dram_tensor("inp", [t_total, F], f32, kind="ExternalInput")
    out = nc.dram_tensor("out", [t_total, U], f32, kind="ExternalOutput")
    ident_d = nc.inline_tensor(np.eye(L, dtype=np.float32), name="ident")
    ident4_d = nc.inline_tensor(
        np.tile(np.eye(L, dtype=np.float32), (1, G)), name="ident4"
    )
    e127c_np = np.zeros((L, 1), dtype=np.float32)
    e127c_np[L - 1, 0] = 1.0
    e127c_d = nc.inline_tensor(e127c_np, name="e127c")

    with tile.TileContext(nc) as tc:
        with (
            tc.tile_pool(name="const", bufs=1) as constp,
            tc.tile_pool(name="inpool", bufs=8) as inpool,
            tc.tile_pool(name="upool", bufs=3) as upool,
            tc.tile_pool(name="mpool", bufs=3) as mpool,
            tc.tile_pool(name="rowpool", bufs=2) as rowpool,
            tc.tile_pool(name="prowpool", bufs=2) as prowpool,
            tc.tile_pool(name="bcpool", bufs=3) as bcpool,
            tc.tile_pool(name="pbcpool", bufs=2) as pbcpool,
            tc.tile_pool(name="selpool", bufs=2) as selpool,
            tc.tile_pool(name="outpool", bufs=4) as outpool,
            tc.tile_pool(name="carrypool", bufs=3) as carrypool,
            tc.tile_pool(name="ps_small", bufs=2, space="PSUM") as ps_small_pool,
            tc.tile_pool(name="ps_out", bufs=3, space="PSUM") as ps_out_pool,
        ):
            ident = constp.tile([L, L], f32, tag="ident")
            nc.sync.dma_start(ident[:], ident_d[:, :])
            ident4 = constp.tile([L, G * L], f32, tag="ident4")
            nc.sync.dma_start(ident4[:], ident4_d[:, :])
            e127c = constp.tile([L, 1], f32, tag="e127c")
            nc.sync.dma_start(e127c[:], e127c_d[:, :])

            prev = None
            tins = {}
            for g in range(ng):
                ks = list(range(g * G, min((g + 1) * G, nb)))
                # per-group y rows: yrow4[0, L*j + i] = y(block ks[j], step i),
                # with the block-start column zeroed (scan state reset)
                yrow4 = rowpool.tile([1, G * L], f32, tag="yrow4")
                nc.vector.memset(yrow4[:], 0.0)
                for j, k in enumerate(ks):
                    r0 = k * L
                    tin = inpool.tile([L, F], f32, tag="tin")
                    nc.sync.dma_start(tin[:], inp[r0 : r0 + L, :])
                    tins[k] = tin
                    ps = ps_small_pool.tile([1, L], f32, tag="ps_small")
                    nc.tensor.transpose(ps[0:1, :], tin[:, 1:2], ident[:])
                    nc.scalar.copy(yrow4[0:1, L * j + 1 : L * j + L], ps[0:1, 1:L])
                ybc4 = bcpool.tile([L, G * L], f32, tag="ybc4")
                nc.gpsimd.partition_broadcast(ybc4[:], yrow4[0:1, :])

                # mt4[s, L*j + t] = M_{ks[j]}[t, s]
                mt4 = mpool.tile([L, G * L], f32r, tag="mt4")
                nc.vector.tensor_tensor_scan(
                    mt4[:],
                    ybc4[:],
                    ident4[:],
                    0.0,
                    mybir.AluOpType.mult,
                    mybir.AluOpType.add,
                )

                for j, k in enumerate(ks):
                    r0 = k * L
                    tin = tins.pop(k)
                    mtk = mt4[:, L * j : L * j + L]

                    # u[s, :] = x_s * z_s
                    u = upool.tile([L, U], f32r, tag="u")
                    nc.scalar.activation(
                        u[:],
                        tin[:, 2:F],
                        mybir.ActivationFunctionType.Copy,
                        scale=tin[:, 0:1],
                    )

                    po = ps_out_pool.tile([L, U], f32, tag="po")
                    if k > 0:
                        # p_t = prod_{r=block_start..t} y_r = y_0 * mt[0, t]
                        prow = prowpool.tile([1, L], f32, tag="prow")
                        nc.vector.tensor_scalar_mul(
                            prow[:], mtk[0:1, :], tin[0:1, 1:2]
                        )
                        pbc = pbcpool.tile([L, L], f32, tag="pbc")
                        nc.gpsimd.partition_broadcast(pbc[:], prow[0:1, :])
                        # sel[s, t] = I[s==127] * p_t
                        sel = selpool.tile([L, L], bf16, tag="sel")
                        nc.vector.tensor_scalar_mul(sel[:], pbc[:], e127c[:])
                    for jj in (0, 512):
                        nc.tensor.matmul(
                            po[:, jj : jj + 512],
                            mtk,
                            u[:, jj : jj + 512],
                            start=True,
                            stop=(k == 0),
                        )
                    if k > 0:
                        # po[t, :] += p_t * prev[127, :]
                        for jj in (0, 512):
                            nc.tensor.matmul(
                                po[:, jj : jj + 512],
                                sel[:],
                                prev[:, jj : jj + 512],
                                start=False,
                                stop=True,
                            )
                    # bf16 carry copy (feeds the next block's rank-1) first,
                    # full-precision output drain second
                    otb = carrypool.tile([L, U], bf16, tag="otb")
                    nc.scalar.copy(otb[:, 0:512], po[:, 0:512])
                    nc.vector.tensor_copy(otb[:, 512:1024], po[:, 512:1024])
                    ot = outpool.tile([L, U], f32r, tag="ot")
                    nc.scalar.copy(ot[:, 0:512], po[:, 0:512])
                    nc.vector.tensor_copy(ot[:, 512:1024], po[:, 512:1024])
                    nc.sync.dma_start(out[r0 : r0 + L, :], ot[:].bitcast(f32))
                    prev = otb
    nc.finalize()
    return nc


_NC = None


def _get_nc() -> bass.Bass:
    global _NC
    if _NC is None:
        _NC = build_nc()
    return _NC


def kernel(**inputs: np.ndarray) -> np.ndarray:
    x = np.ascontiguousarray(inputs["inputs"], dtype=np.float32)
    assert x.shape == (B, T, F), x.shape
    nc = _get_nc()
    in_maps = [{"inp": x[c]} for c in range(B)]
    res = run_bass_kernel_spmd(nc, in_maps, core_ids=list(range(B)))
    return np.stack([res.results[c]["out"] for c in range(B)], axis=0)



# revision 11
# speedup vs baseline: 1.4354x; 1.4354x over previous
"""Trainium2 Bass kernel for CLSProcess: diagonal linear recurrence
state_t = y_t * state_{t-1} + x_t * z_t over [B=8, T=4096, units=1024].

Sharding: batch across the 8 cores (one batch element per core); the
recurrence is handled per-core with a chunked scan:
  - time is cut into 32 blocks of L=128 steps (partition dim = time)
  - per block, the decay matrix M'[t,s] = x_s * prod_{r=s+1..t} y_r
    (0 for s>t) is built EXACTLY with a DVE tensor_tensor_scan over the
    identity: state_s(t) = y_t*state + I[s==t]  =>  out[s,t] = M[t,s]
    (the lhsT layout the PE matmul wants), then scaled per-partition by
    x_s (x is staged time-major in the partition dim by the host).
    Scans are batched 4 blocks per instruction ([128,512]) with the y
    at block boundaries zeroed so the running state resets at each
    block start.
  - block output = M' @ z  (PE matmul, bf16 operands, fp32 PSUM) +
    carry term
  - carry term: po[t,:] += p_t * prev[127,:] with
    p_t = prod_{r=block_start..t} y_r = y_block_start * mt4raw[0, t]
    (no separate scan needed). Engines can only address partition
    bases {0,32,64,96} and matmul bases {0,32,64}, so row 127 of the
    previous block is reached with a K=64 matmul: sel[s,t] =
    I[s==127] * p_t (rows [64:128] used) against prev[64:128,:].
  - engine budget (cost = free-dim size x cycle): PSUM drains are
    split by the free dim across the scalar and vector engines; the
    small per-partition scale multiplies (x fold-in, sel mask) run on
    the scalar engine's activation-scale path.
  - I/O is bf16 (z in, out), halving HBM traffic; x/y ride along as a
    small fp32 sidecar so the decay products stay full precision. The
    host upcasts the bf16 result to fp32.
"""

import numpy as np
import ml_dtypes

import concourse.bacc as bacc
import concourse.bass as bass
import concourse.mybir as mybir
import concourse.tile as tile
from concourse.bass_utils import run_bass_kernel_spmd

B = 8
T = 4096
F = 1026
U = 1024
L = 128
G = 4  # blocks per scan batch
NB = T // L  # 32 blocks
NG = NB // G  # 8 groups
CW = 64  # carry matmul contraction width (matmul bases must be 0/32/64)
f32 = mybir.dt.float32
bf16 = mybir.dt.bfloat16


def build_nc() -> bass.Bass:
    nc = bacc.Bacc()
    zin = nc.dram_tensor("zin", [T, U], bf16, kind="ExternalInput")
    # yz[0, t] = y_t with block-start entries zeroed (scan reset)
    yz = nc.dram_tensor("yz", [1, T], f32, kind="ExternalInput")
    # xct[p, k] = x_{k*L+p}: x staged with time in the partition dim
    xct = nc.dram_tensor("xct", [L, NB], f32, kind="ExternalInput")
    # e127y[p, k] = I[p==127] * y_{k*L}: sel mask+scale per block
    e127y = nc.dram_tensor("e127y", [L, NB], f32, kind="ExternalInput")
    out = nc.dram_tensor("out", [T, U], bf16, kind="ExternalOutput")

    ident4_d = nc.inline_tensor(
        np.tile(np.eye(L, dtype=np.float32), (1, G)), name="ident4"
    )

    with tile.TileContext(nc) as tc:
        with (
            tc.tile_pool(name="const", bufs=1) as constp,
            tc.tile_pool(name="zpool", bufs=10) as zpool,
            tc.tile_pool(name="mtraw", bufs=2) as mtrawp,
            tc.tile_pool(name="mtpool", bufs=2) as mtp,
            tc.tile_pool(name="ybcpool", bufs=2) as ybcp,
            tc.tile_pool(name="pbcpool", bufs=2) as pbcp,
            tc.tile_pool(name="selpool", bufs=2) as selp,
            tc.tile_pool(name="otpool", bufs=6) as otp,
            tc.tile_pool(name="ps_out", bufs=3, space="PSUM") as psp,
        ):
            ident4 = constp.tile([L, G * L], f32, tag="ident4")
            nc.sync.dma_start(ident4[:], ident4_d[:, :])
            xcol = constp.tile([L, NB], f32, tag="xcol")
            nc.sync.dma_start(xcol[:], xct[:, :])
            ecol = constp.tile([L, NB], f32, tag="ecol")
            nc.sync.dma_start(ecol[:], e127y[:, :])
            yzfull = constp.tile([1, T], f32, tag="yz")
            nc.sync.dma_start(yzfull[:], yz[0:1, :])

            prev = None
            for g in range(NG):
                c0 = g * G * L
                ybc4 = ybcp.tile([L, G * L], f32, tag="ybc4")
                nc.gpsimd.partition_broadcast(ybc4[:], yzfull[0:1, c0 : c0 + G * L])

                # mt4raw[s, L*j + t] = prod_{r=s+1..t} y_r  (block g*G+j)
                mt4raw = mtrawp.tile([L, G * L], f32, tag="mt4raw")
                nc.vector.tensor_tensor_scan(
                    mt4raw[:],
                    ybc4[:],
                    ident4[:],
                    0.0,
                    mybir.AluOpType.mult,
                    mybir.AluOpType.add,
                )
                # pbc4[p, L*j + t] = mt4raw[0, L*j + t]  (broadcast row 0)
                pbc4 = pbcp.tile([L, G * L], f32, tag="pbc4")
                nc.gpsimd.partition_broadcast(pbc4[:], mt4raw[0:1, :])

                # fold x in: mt4[s, L*j+t] = x_s * mt4raw[s, L*j+t] and
                # sel4[s, L*j+t] = I[s==127] * y0_k * mt4raw[0, L*j+t]
                # (= I[s==127] * p_t), via activation-scale on Scalar
                mt4 = mtp.tile([L, G * L], bf16, tag="mt4")
                sel4 = selp.tile([L, G * L], bf16, tag="sel4")
                for j in range(G):
                    k = g * G + j
                    nc.scalar.activation(
                        mt4[:, L * j : L * (j + 1)],
                        mt4raw[:, L * j : L * (j + 1)],
                        mybir.ActivationFunctionType.Copy,
                        scale=xcol[:, k : k + 1],
                    )
                    if k > 0:
                        nc.scalar.activation(
                            sel4[:, L * j : L * (j + 1)],
                            pbc4[:, L * j : L * (j + 1)],
                            mybir.ActivationFunctionType.Copy,
                            scale=ecol[:, k : k + 1],
                        )

                for j in range(G):
                    k = g * G + j
                    r0 = k * L
                    tz = zpool.tile([L, U], bf16, tag="tz")
                    nc.sync.dma_start(tz[:], zin[r0 : r0 + L, :])

                    po = psp.tile([L, U], f32, tag="po")
                    for jj in (0, 512):
                        nc.tensor.matmul(
                            po[:, jj : jj + 512],
                            mt4[:, L * j : L * (j + 1)],
                            tz[:, jj : jj + 512],
                            start=True,
                            stop=(k == 0),
                        )
                    if k > 0:
                        # po[t, :] += p_t * prev[127, :]
                        for jj in (0, 512):
                            nc.tensor.matmul(
                                po[:, jj : jj + 512],
                                sel4[L - CW : L, L * j : L * (j + 1)],
                                prev[L - CW : L, jj : jj + 512],
                                start=False,
                                stop=True,
                            )
                    ot = otp.tile([L, U], bf16, tag="ot")
                    # drain split by the free dim: one half per engine
                    nc.scalar.copy(ot[:, 0:512], po[:, 0:512])
                    nc.vector.tensor_copy(ot[:, 512:1024], po[:, 512:1024])
                    nc.sync.dma_start(out[r0 : r0 + L, :], ot[:])
                    prev = ot
    nc.finalize()
    return nc


_NC = None


def _get_nc() -> bass.Bass:
    global _NC
    if _NC is None:
        _NC = build_nc()
    return _NC


def prep_in_maps(x: np.ndarray) -> list[dict]:
    xs = x[:, :, 0]  # [B,T]
    ys = x[:, :, 1]  # [B,T]
    zb = np.ascontiguousarray(x[:, :, 2:]).astype(ml_dtypes.bfloat16)

    mask0 = (np.arange(T) % L) == 0
    yz = np.where(mask0[None, :], 0.0, ys).astype(np.float32)[:, None, :]
    xct = np.ascontiguousarray(xs.reshape(B, NB, L).transpose(0, 2, 1))
    e127y = np.zeros((B, L, NB), dtype=np.float32)
    e127y[:, L - 1, :] = ys[:, ::L]

    return [
        {"zin": zb[c], "yz": yz[c], "xct": xct[c], "e127y": e127y[c]}
        for c in range(B)
    ]


def kernel(**inputs: np.ndarray) -> np.ndarray:
    x = np.ascontiguousarray(inputs["inputs"], dtype=np.float32)
    assert x.shape == (B, T, F), x.shape
    nc = _get_nc()
    res = run_bass_kernel_spmd(nc, prep_in_maps(x), core_ids=list(range(B)))
    outb = np.stack([res.results[c]["out"] for c in range(B)], axis=0)
    return outb.astype(np.float32)
